# revision 54
# baseline (speedup 1.0000x reference)
"""LEGOTransformer (moe_routing early-exit) Trainium2 Bass kernel.

Reference semantics: tokens run through block0 (layers 0,1), compute
logits0 = hb0 @ head_w.T; tokens whose max softmax prob >= 1e-4 exit and
keep logits0. Remaining tokens run block1 (layers 2,3) from hb0 and take
logits1 (last block always writes active tokens).

Single fused launch, fully token-sharded (512 tok/core, no collectives):

  Layers: embedding rows -> 2 transformer layers, feature-major
    activations ([D, tok] in SBUF). LN scale/bias are folded into the
    weights host-side (wv' = s1*wv, w1' = s2*w1, c_att = (ln1_b@wv)@wo,
    b1' = b1 + ln2_b@w1), so the device LN only computes
    x_hat = (x-mu)*rstd via per-token rows A=rstd, B=-mu*rstd (broadcast
    across partitions with two K=1 matmuls). Stats come from bf16 shadow
    copies hr=h, hsq=h*h maintained in the matmul epilogues, so the
    stats matmuls run at full PE rate and are ready immediately.
    Stats/chain are split into token halves so the vector chain of one
    half hides under the other half's tensor work (keeps the PE p-state
    at max clock). Weights and matmul activations are bf16 (full PE
    rate, half the DMA); the residual stream h stays fp32.

  Head: each core computes logits for its OWN 512 tokens over the FULL
    vocab (padded to 50304 = 393*128), reusing the bf16 shadow hr as the
    moving operand: out[vocab128, tok] tiles, written bf16 to DRAM in
    vocab-major layout [50304, 512]; the host transposes/casts when
    assembling the [T, VOCAB] fp32 output. No cross-core collective and
    no second launch: the head stream starts as soon as the last layer's
    epilogues produce hr.

  Host: max-softmax exit mask computed from the full logits on host
    (identical decision to reference's max softmax >= 1e-4); tokens that
    do not exit (none for this input distribution, but handled honestly)
    get block1 + their logits row recomputed on host in fp32 numpy and
    patched in.
"""

import sys

sys.path.insert(0, "/opt/trn_rl_repo")

from contextlib import ExitStack

import ml_dtypes
import numpy as np

from concourse import bacc, tile, mybir
from concourse.bass_utils import run_bass_kernel_spmd

F32 = mybir.dt.float32
F32R = mybir.dt.float32r
BF16 = mybir.dt.bfloat16
AF = mybir.ActivationFunctionType
OP = mybir.AluOpType

VOCAB = 50257
DIM = 1024
DFF = 4096
T = 4096
NCORES = 8
TPC = T // NCORES          # tokens per core
NVG = 393                  # vocab 128-tiles (393*128 = 50304 >= 50257)
VP2 = NVG * 128
LN_EPS = 1e-5
MHAT = 16.0                # fixed exp shift for host softmax stats
THRESH = 1e-4
HALVES = (slice(0, TPC // 2), slice(TPC // 2, TPC))

_cache = {}

# test-harness knobs (harness never touches these; defaults are production)
TRACE = False
LAST_EXEC_NS = {}
LAST_PROFILE = {}


# --------------------------------------------------------------------------
# Fused launch: two transformer layers + full-vocab head, token-sharded
# --------------------------------------------------------------------------

def _build_F():
    nc = bacc.Bacc(None, target_bir_lowering=False)
    hT = nc.declare_dram_parameter("hT", [DIM, TPC], F32, isOutput=False)
    wvf = nc.declare_dram_parameter("wvf", [2, DIM, DIM], BF16, isOutput=False)
    wof = nc.declare_dram_parameter("wof", [2, DIM, DIM], BF16, isOutput=False)
    w1f = nc.declare_dram_parameter("w1f", [2, DIM, DFF], BF16, isOutput=False)
    w2f = nc.declare_dram_parameter("w2f", [2, DFF, DIM], BF16, isOutput=False)
    b1d = nc.declare_dram_parameter("b1ft", [2, 128, DFF // 128], F32, isOutput=False)
    b2d = nc.declare_dram_parameter("b2t", [2, 128, DIM // 128], F32, isOutput=False)
    cattd = nc.declare_dram_parameter("cattt", [2, 128, DIM // 128], F32, isOutput=False)
    hwTd = nc.declare_dram_parameter("hwT", [DIM, VP2], BF16, isOutput=False)
    hbT = nc.declare_dram_parameter("hbT", [DIM, TPC], F32, isOutput=True)
    logT = nc.declare_dram_parameter("logT", [VP2, TPC], BF16, isOutput=True)

    with tile.TileContext(nc) as tc, ExitStack() as ctx:
        p_h = ctx.enter_context(tc.tile_pool(name="p_h", bufs=1))
        p_hr = ctx.enter_context(tc.tile_pool(name="p_hr", bufs=1))
        p_act = ctx.enter_context(tc.tile_pool(name="p_act", bufs=2))
        p_tmp = ctx.enter_context(tc.tile_pool(name="p_tmp", bufs=1))
        p_g = ctx.enter_context(tc.tile_pool(name="p_g", bufs=1))
        p_w = ctx.enter_context(tc.tile_pool(name="p_w", bufs=14))
        p_wB = ctx.enter_context(tc.tile_pool(name="p_wB", bufs=6))
        p_lo = ctx.enter_context(tc.tile_pool(name="p_lo", bufs=3))
        p_st = ctx.enter_context(tc.tile_pool(name="p_st", bufs=2))
        p_c = ctx.enter_context(tc.tile_pool(name="p_c", bufs=1))
        p_mm = ctx.enter_context(tc.tile_pool(name="p_mm", bufs=5, space="PSUM"))
        p_bc = ctx.enter_context(tc.tile_pool(name="p_bc", bufs=2, space="PSUM"))
        p_s12 = ctx.enter_context(tc.tile_pool(name="p_s12", bufs=1, space="PSUM"))

        ones128f = p_c.tile([128, 1], F32, tag="ones128f")
        nc.gpsimd.memset(ones128f[:], 1.0)
        ones128b = p_c.tile([128, 1], BF16, tag="ones128b")
        nc.vector.tensor_copy(ones128b[:], ones128f[:])
        eps_t = p_c.tile([1, 1], F32, tag="eps")
        nc.gpsimd.memset(eps_t[:], LN_EPS)
        rowf = p_c.tile([1, 128], F32, tag="rowf")
        nc.gpsimd.memset(rowf[:], 1.0)
        onesrow = p_c.tile([1, 128], F32R, tag="onesrow")  # stationary for bcasts
        nc.vector.tensor_copy(onesrow[:], rowf[:])


        h_fm = p_h.tile([128, 8, TPC], F32, tag="h")
        hr = p_hr.tile([128, 8, TPC], BF16, tag="hr")
        hsq = p_hr.tile([128, 8, TPC], BF16, tag="hsq")
        # chunked load + immediate bf16 shadow prep (per k, per half);
        # issued before the bias loads so the first LN isn't queued behind them
        for k in range(8):
            for ci, cs in enumerate(HALVES):
                eng = nc.gpsimd if (k * 2 + ci) % 2 == 0 else nc.sync
                eng.dma_start(h_fm[:, k, cs], hT[k * 128 : (k + 1) * 128, cs])
                nc.vector.tensor_copy(hr[:, k, cs], h_fm[:, k, cs])
                nc.scalar.activation(hsq[:, k, cs], h_fm[:, k, cs], AF.Square)

        b1_sb = {}
        b2_sb = {}
        catt_sb = {}
        for li in range(2):
            t1 = p_c.tile([128, DFF // 128], F32, tag=f"b1_{li}")
            nc.sync.dma_start(t1[:], b1d[li])
            b1_sb[li] = t1
            t2 = p_c.tile([128, DIM // 128], F32, tag=f"b2_{li}")
            nc.sync.dma_start(t2[:], b2d[li])
            b2_sb[li] = t2
            t3 = p_c.tile([128, DIM // 128], F32, tag=f"catt_{li}")
            nc.sync.dma_start(t3[:], cattd[li])
            catt_sb[li] = t3

        def update_shadow(m, last=False):
            """After h_fm[:, m, :] residual update: refresh hr (+hsq/store).

            hr cast on vector right behind the residual stt; hsq on scalar."""
            nc.vector.tensor_copy(hr[:, m, :], h_fm[:, m, :])
            if last:
                for cs in HALVES:
                    nc.sync.dma_start(hbT[m * 128 : (m + 1) * 128, cs], h_fm[:, m, cs])
            else:
                nc.scalar.activation(hsq[:, m, :], h_fm[:, m, :], AF.Square)

        def emit_ln():
            """x_hat = (h - mu) * rstd -> returns bf16 act tile [128, 8, TPC]."""
            s12 = p_s12.tile([33, TPC], F32, tag="s12")
            for cs in HALVES:
                for k in range(8):
                    nc.tensor.matmul(
                        s12[0:1, cs], ones128b[:], hr[:, k, cs],
                        start=(k == 0), stop=(k == 7),
                    )
                for k in range(8):
                    nc.tensor.matmul(
                        s12[32:33, cs], ones128b[:], hsq[:, k, cs],
                        start=(k == 0), stop=(k == 7),
                    )
            mu_t = p_st.tile([1, TPC], F32, tag="mu")
            var_t = p_st.tile([1, TPC], F32, tag="var")
            rstd_t = p_st.tile([1, TPC], F32R, tag="rstd")
            bt_t = p_st.tile([1, TPC], F32R, tag="bt")
            abA = p_bc.tile([128, TPC], F32, tag="bc", name="abA")
            bbB = p_bc.tile([128, TPC], F32, tag="bc", name="bbB")
            # bf16 copies of the broadcast rows: the apply then runs all-16-bit
            # on the DVE (2x throughput) reading the bf16 shadow hr
            abA_b = p_st.tile([128, TPC], BF16, tag="abA_b")
            bbB_b = p_st.tile([128, TPC], BF16, tag="bbB_b")
            with nc.allow_low_precision(reason="LN rows feed f32r matmuls"):
                for cs in HALVES:
                    nc.vector.tensor_scalar_mul(mu_t[:, cs], s12[0:1, cs], 1.0 / DIM)
                    # E[x^2] on the scalar engine, in parallel with mu
                    nc.scalar.activation(
                        var_t[:, cs], s12[32:33, cs], AF.Copy, scale=1.0 / DIM
                    )
                    musq = p_st.tile([1, TPC], F32, tag="musq")
                    nc.vector.scalar_tensor_tensor(
                        musq[:, cs], mu_t[:, cs], -1.0, mu_t[:, cs], OP.mult, OP.mult
                    )
                    nc.vector.tensor_add(var_t[:, cs], var_t[:, cs], musq[:, cs])
                    # var+eps > 0, so 1/sqrt(|x|) == rsqrt
                    nc.scalar.activation(
                        rstd_t[:, cs], var_t[:, cs], AF.Abs_reciprocal_sqrt,
                        bias=eps_t[:], scale=1.0,
                    )
                    nc.vector.scalar_tensor_tensor(
                        bt_t[:, cs], mu_t[:, cs], -1.0, rstd_t[:, cs],
                        OP.mult, OP.mult,
                    )
                    nc.tensor.matmul(
                        abA[:, cs], onesrow[:], rstd_t[:, cs], start=True, stop=True
                    )
                    nc.tensor.matmul(
                        bbB[:, cs], onesrow[:], bt_t[:, cs], start=True, stop=True
                    )
                    # casts split across engines so they overlap
                    nc.scalar.activation(abA_b[:, cs], abA[:, cs], AF.Copy)
                    nc.vector.tensor_copy(bbB_b[:, cs], bbB[:, cs])
            dst = p_act.tile([128, 8, TPC], BF16, tag="act")
            with nc.allow_low_precision(reason="bf16 matmul inputs"):
                # h0 columns first: the next stream's split first group can
                # start its h0-column matmuls while h1's chain still runs
                for cs in HALVES:
                    for k in range(8):
                        nc.vector.tensor_mul(dst[:, k, cs], hr[:, k, cs], abA_b[:, cs])
                        nc.vector.tensor_add(dst[:, k, cs], dst[:, k, cs], bbB_b[:, cs])
            return dst

        def matmul_stream(src_fm, wdram, kt, mt, epilogue,
                          groups=None, m_major_last=False, split_first_cols=False):
            """dst[m] = sum_k w[k,m].T @ src[k], feature-major, full 512 moving.

            src_fm: [128, kt, TPC] bf16; wdram: [kt*128, mt*128] bf16.
            epilogue(m, acc) consumes the accumulated PSUM tile.
            groups: m-tile group sizes (default fours). m_major_last runs the
            last group m-at-a-time so its epilogues stagger instead of all
            releasing at stream end (keeps the next LN's stats fed).
            """
            if groups is None:
                groups = [4] * (mt // 4) + ([mt % 4] if mt % 4 else [])
            m0 = 0
            for gi, gsz in enumerate(groups):
                w_ = gsz * 128
                m_major = m_major_last and gi == len(groups) - 1 and kt <= 8
                if gi == 0 and split_first_cols and kt <= 8:
                    # half-width column passes (h0 then h1) as SEQUENTIAL
                    # PSUM region groups in the same banks: the h0 pass only
                    # needs half the LN apply, hiding the h1 chain tail
                    wts = []
                    for k in range(kt):
                        wt = p_w.tile([128, 512], BF16, tag="wt")
                        nc.sync.dma_start(
                            wt[:, :w_],
                            wdram[k * 128 : (k + 1) * 128, m0 * 128 : m0 * 128 + w_],
                        )
                        wts.append(wt)
                    accs = {
                        ml: p_mm.tile([128, TPC], F32, tag="mm", name=f"acc{ml}")
                        for ml in range(gsz)
                    }
                    for cs in HALVES:
                        for k in range(kt):
                            for ml in range(gsz):
                                nc.tensor.matmul(
                                    accs[ml][:, cs],
                                    wts[k][:, ml * 128 : (ml + 1) * 128],
                                    src_fm[:, k, cs],
                                    start=(k == 0),
                                    stop=(k == kt - 1),
                                )
                    for ml in range(gsz):
                        epilogue(m0 + ml, accs[ml])
                    m0 += gsz
                    continue
                if m_major:
                    wts = []
                    for k in range(kt):
                        wt = p_w.tile([128, 512], BF16, tag="wt")
                        nc.sync.dma_start(
                            wt[:, :w_],
                            wdram[k * 128 : (k + 1) * 128, m0 * 128 : m0 * 128 + w_],
                        )
                        wts.append(wt)
                    for ml in range(gsz):
                        acc = p_mm.tile([128, TPC], F32, tag="mm", name=f"acc{ml}")
                        for k in range(kt):
                            nc.tensor.matmul(
                                acc[:],
                                wts[k][:, ml * 128 : (ml + 1) * 128],
                                src_fm[:, k, :],
                                start=(k == 0),
                                stop=(k == kt - 1),
                            )
                        epilogue(m0 + ml, acc)
                else:
                    accs = {}
                    for k in range(kt):
                        wt = p_w.tile([128, 512], BF16, tag="wt")
                        nc.sync.dma_start(
                            wt[:, :w_],
                            wdram[k * 128 : (k + 1) * 128, m0 * 128 : m0 * 128 + w_],
                        )
                        for ml in range(gsz):
                            if k == 0:
                                accs[ml] = p_mm.tile(
                                    [128, TPC], F32, tag="mm", name=f"acc{ml}"
                                )
                            nc.tensor.matmul(
                                accs[ml][:],
                                wt[:, ml * 128 : (ml + 1) * 128],
                                src_fm[:, k, :],
                                start=(k == 0),
                                stop=(k == kt - 1),
                            )
                    for ml in range(gsz):
                        epilogue(m0 + ml, accs[ml])
                m0 += gsz

        for li in range(2):
            # --- attention (seq len 1): h += LN1(h) @ wv' @ wo + c_att ---
            a_fm = emit_ln()
            tmp_fm = p_tmp.tile([128, 8, TPC], BF16, tag="tmp")

            def ep_tmp(m, acc):
                nc.vector.tensor_copy(tmp_fm[:, m, :], acc[:])

            matmul_stream(a_fm, wvf[li], 8, 8, ep_tmp, split_first_cols=True)

            def ep_resid_att(m, acc, li=li):
                nc.vector.scalar_tensor_tensor(
                    h_fm[:, m, :], acc[:], catt_sb[li][:, m : m + 1], h_fm[:, m, :],
                    OP.add, OP.add,
                )
                update_shadow(m)

            matmul_stream(tmp_fm, wof[li], 8, 8, ep_resid_att)

            # --- mlp: h += gelu(LN2(h) @ w1' + b1') @ w2 + b2 ---
            m_fm = emit_ln()
            g_fm = p_g.tile([128, 32, TPC], BF16, tag="g")

            def ep_gelu(m, acc, li=li):
                nc.scalar.activation(
                    g_fm[:, m, :],
                    acc[:],
                    AF.Gelu_apprx_tanh,
                    bias=b1_sb[li][:, m : m + 1],
                    scale=1.0,
                )

            matmul_stream(m_fm, w1f[li], 8, 32, ep_gelu, split_first_cols=True)

            last = li == 1

            def ep_resid_mlp(m, acc, li=li, last=last):
                nc.vector.scalar_tensor_tensor(
                    h_fm[:, m, :], acc[:], b2_sb[li][:, m : m + 1], h_fm[:, m, :],
                    OP.add, OP.add,
                )
                update_shadow(m, last=last)

            matmul_stream(g_fm, w2f[li], 32, 8, ep_resid_mlp)

        # --- head: logits[v, t] = head_w[v, :] @ hb[:, t], full vocab ---
        for mg in range((NVG + 3) // 4):
            mls = [ml for ml in range(4) if mg * 4 + ml < NVG]
            w_ = len(mls) * 128
            wtbs = []
            for kc in range(2):
                wtb = p_wB.tile([128, 4, 512], BF16, tag="wtb")
                nc.sync.dma_start(
                    wtb[:, :, :w_],
                    hwTd[
                        kc * 512 : (kc + 1) * 512, mg * 512 : mg * 512 + w_
                    ].rearrange("(k p) v -> p k v", p=128),
                )
                wtbs.append(wtb)
            accs = {}
            for k in range(8):
                wtb = wtbs[k // 4]
                for ml in mls:
                    if k == 0:
                        accs[ml] = p_mm.tile([128, TPC], F32, tag="mm", name=f"ha{ml}")
                    nc.tensor.matmul(
                        accs[ml][:],
                        wtb[:, k % 4, ml * 128 : (ml + 1) * 128],
                        hr[:, k, :],
                        start=(k == 0),
                        stop=(k == 7),
                    )
            lo = p_lo.tile([128, 4, TPC], BF16, tag="lo")
            with nc.allow_low_precision(reason="bf16 logits output"):
                for ml in mls:
                    nc.vector.tensor_copy(lo[:, ml, :], accs[ml][:])
            if mg >= (NVG + 3) // 4 - 3:
                # near the end: per-vocab-tile (and, for the last two groups,
                # per-token-half) stores spread the drain across queues so the
                # kernel doesn't tail-wait on one big DMA
                fine = mg >= (NVG + 3) // 4 - 2
                for ml in mls:
                    vg = mg * 4 + ml
                    if fine:
                        for cs in HALVES:
                            nc.sync.dma_start(
                                logT[vg * 128 : (vg + 1) * 128, cs], lo[:, ml, cs]
                            )
                    else:
                        nc.sync.dma_start(
                            logT[vg * 128 : (vg + 1) * 128, :], lo[:, ml, :]
                        )
            else:
                nc.sync.dma_start(
                    logT[mg * 512 : mg * 512 + w_, :].rearrange(
                        "(g p) t -> p g t", p=128
                    ),
                    lo[:, : len(mls), :],
                )

    nc.compile()
    return nc


def _get():
    if "F" not in _cache:
        _cache["F"] = _build_F()
    return _cache["F"]


# --------------------------------------------------------------------------
# Host side
# --------------------------------------------------------------------------

def _gelu_tanh(x):
    return 0.5 * x * (1.0 + np.tanh(0.7978845608028654 * (x + 0.044715 * x * x * x)))


def _host_block1(hb, inputs):
    """Block-1 layers (li=2,3) + head, fp32 numpy, for non-exiting tokens."""
    hb = hb.astype(np.float32)
    for li in (2, 3):
        mu = hb.mean(-1, keepdims=True, dtype=np.float32)
        var = hb.var(-1, keepdims=True, dtype=np.float32)
        a = (hb - mu) / np.sqrt(var + LN_EPS)
        a = a * inputs["ln1_s"][li] + inputs["ln1_b"][li]
        hb = hb + (a @ inputs["wv"][li]) @ inputs["wo"][li]
        mu = hb.mean(-1, keepdims=True, dtype=np.float32)
        var = hb.var(-1, keepdims=True, dtype=np.float32)
        m = (hb - mu) / np.sqrt(var + LN_EPS)
        m = m * inputs["ln2_s"][li] + inputs["ln2_b"][li]
        hb = hb + _gelu_tanh(m @ inputs["w1"][li] + inputs["b1"][li]) @ inputs["w2"][
            li
        ] + inputs["b2"][li]
    return hb @ np.asarray(inputs["head_w"], np.float32).T


def kernel(**inputs):
    x = np.asarray(inputs["x"]).reshape(-1).astype(np.int64)
    emb = np.asarray(inputs["emb"], dtype=np.float32)
    head_w = np.asarray(inputs["head_w"], dtype=np.float32)
    f32c = lambda k: np.ascontiguousarray(np.asarray(inputs[k], dtype=np.float32))

    h0 = emb[x]  # [T, DIM]

    wv = f32c("wv")[:2]
    wo = f32c("wo")[:2]
    w1 = f32c("w1")[:2]
    w2 = f32c("w2")[:2]
    ln1s, ln1b = f32c("ln1_s")[:2], f32c("ln1_b")[:2]
    ln2s, ln2b = f32c("ln2_s")[:2], f32c("ln2_b")[:2]
    b1, b2 = f32c("b1")[:2], f32c("b2")[:2]

    bf = lambda a: np.ascontiguousarray(a).astype(ml_dtypes.bfloat16)
    wvf = bf(ln1s[:, :, None] * wv)                       # fold ln1 scale
    wof = bf(wo)
    w1f = bf(ln2s[:, :, None] * w1)                       # fold ln2 scale
    w2f = bf(w2)
    catt = np.einsum("ld,ldm->lm", ln1b, wv, optimize=True)
    catt = np.einsum("ld,ldm->lm", catt, wo, optimize=True).astype(np.float32)
    b1f = (b1 + np.einsum("ld,ldm->lm", ln2b, w1, optimize=True)).astype(np.float32)
    # pre-transposed per-partition bias layouts: [L, 128, M]
    tp = lambda a, m: np.ascontiguousarray(
        a.reshape(2, m, 128).transpose(0, 2, 1).astype(np.float32)
    )
    hwT = np.zeros((DIM, VP2), ml_dtypes.bfloat16)
    hwT[:, :VOCAB] = head_w.T.astype(ml_dtypes.bfloat16)

    ncF = _get()
    wF = {
        "wvf": wvf, "wof": wof, "w1f": w1f, "w2f": w2f,
        "b1ft": tp(b1f, DFF // 128), "b2t": tp(b2, DIM // 128),
        "cattt": tp(catt, DIM // 128), "hwT": hwT,
    }
    in_maps = []
    for c in range(NCORES):
        m = dict(wF)
        m["hT"] = np.ascontiguousarray(h0[c * TPC : (c + 1) * TPC].T)
        in_maps.append(m)
    res = run_bass_kernel_spmd(
        ncF, in_maps, core_ids=list(range(NCORES)), trace=TRACE
    )
    if TRACE:
        LAST_EXEC_NS["F"] = res.exec_time_ns
        LAST_PROFILE["F"] = res

    out = np.empty((T, VOCAB), np.float32)
    for c in range(NCORES):
        L = res.results[c]["logT"]  # [VP2, TPC] bf16
        out[c * TPC : (c + 1) * TPC, :] = L[:VOCAB].T.astype(np.float32)
    hbT = np.concatenate(
        [res.results[c]["hbT"] for c in range(NCORES)], axis=1
    )  # [DIM, T]

    # host softmax stats (chunked): max_prob = exp(M - MHAT) / sum exp(l - MHAT)
    M = np.empty(T, np.float32)
    Z = np.empty(T, np.float32)
    for i in range(0, T, 256):
        chunk = out[i : i + 256]
        M[i : i + 256] = chunk.max(1)
        Z[i : i + 256] = np.exp(chunk - MHAT, dtype=np.float32).sum(
            1, dtype=np.float32
        )
    max_prob = np.exp(M - MHAT).astype(np.float32) / Z
    cont = ~(max_prob >= THRESH)
    if cont.any():
        idx = np.where(cont)[0]
        out[idx] = _host_block1(hbT.T[idx], inputs)

    return out.reshape(tuple(np.asarray(inputs["x"]).shape) + (VOCAB,))


# revision 58
# speedup vs baseline: 1.0400x; 1.0400x over previous
"""LEGOTransformer (moe_routing early-exit) Trainium2 Bass kernel.

Reference semantics: tokens run through block0 (layers 0,1), compute
logits0 = hb0 @ head_w.T; tokens whose max softmax prob >= 1e-4 exit and
keep logits0. Remaining tokens run block1 (layers 2,3) from hb0 and take
logits1 (last block always writes active tokens).

Single fused launch, fully token-sharded (512 tok/core, no collectives):

  Layers: embedding rows -> 2 transformer layers, feature-major
    activations ([D, tok] in SBUF). LN scale/bias are folded into the
    weights host-side (wv' = s1*wv, w1' = s2*w1, c_att = (ln1_b@wv)@wo,
    b1' = b1 + ln2_b@w1), so the device LN only computes
    x_hat = (x-mu)*rstd via per-token rows A=rstd, B=-mu*rstd (broadcast
    across partitions with two K=1 matmuls). Stats come from bf16 shadow
    copies hr=h, hsq=h*h maintained in the matmul epilogues, so the
    stats matmuls run at full PE rate and are ready immediately.
    Stats/chain are split into token halves so the vector chain of one
    half hides under the other half's tensor work (keeps the PE p-state
    at max clock). Weights and matmul activations are bf16 (full PE
    rate, half the DMA); the residual stream h stays fp32.

  Head: each core computes logits for its OWN 512 tokens over the FULL
    vocab (padded to 50304 = 393*128), reusing the bf16 shadow hr as the
    moving operand: out[vocab128, tok] tiles, written bf16 to DRAM in
    vocab-major layout [50304, 512]; the host transposes/casts when
    assembling the [T, VOCAB] fp32 output. No cross-core collective and
    no second launch: the head stream starts as soon as the last layer's
    epilogues produce hr.

  Host: max-softmax exit mask computed from the full logits on host
    (identical decision to reference's max softmax >= 1e-4); tokens that
    do not exit (none for this input distribution, but handled honestly)
    get block1 + their logits row recomputed on host in fp32 numpy and
    patched in.
"""

import sys

sys.path.insert(0, "/opt/trn_rl_repo")

from contextlib import ExitStack

import ml_dtypes
import numpy as np

from concourse import bacc, tile, mybir
from concourse.bass_utils import run_bass_kernel_spmd

F32 = mybir.dt.float32
F32R = mybir.dt.float32r
BF16 = mybir.dt.bfloat16
AF = mybir.ActivationFunctionType
OP = mybir.AluOpType

VOCAB = 50257
DIM = 1024
DFF = 4096
T = 4096
NCORES = 8
TPC = T // NCORES          # tokens per core
NVG = 393                  # vocab 128-tiles (393*128 = 50304 >= 50257)
VP2 = NVG * 128
LN_EPS = 1e-5
MHAT = 16.0                # fixed exp shift for host softmax stats
THRESH = 1e-4
HALVES = (slice(0, TPC // 2), slice(TPC // 2, TPC))

_cache = {}

# test-harness knobs (harness never touches these; defaults are production)
TRACE = False
LAST_EXEC_NS = {}
LAST_PROFILE = {}


# --------------------------------------------------------------------------
# Fused launch: two transformer layers + full-vocab head, token-sharded
# --------------------------------------------------------------------------

def _build_F():
    nc = bacc.Bacc(None, target_bir_lowering=False)
    hT = nc.declare_dram_parameter("hT", [DIM, TPC], F32, isOutput=False)
    wvf = nc.declare_dram_parameter("wvf", [2, DIM, DIM], BF16, isOutput=False)
    wof = nc.declare_dram_parameter("wof", [2, DIM, DIM], BF16, isOutput=False)
    w1f = nc.declare_dram_parameter("w1f", [2, DIM, DFF], BF16, isOutput=False)
    w2f = nc.declare_dram_parameter("w2f", [2, DFF, DIM], BF16, isOutput=False)
    b1d = nc.declare_dram_parameter("b1ft", [2, 128, DFF // 128], F32, isOutput=False)
    b2d = nc.declare_dram_parameter("b2t", [2, 128, DIM // 128], F32, isOutput=False)
    cattd = nc.declare_dram_parameter("cattt", [2, 128, DIM // 128], F32, isOutput=False)
    hwTd = nc.declare_dram_parameter("hwT", [DIM, VP2], BF16, isOutput=False)
    hbT = nc.declare_dram_parameter("hbT", [DIM, TPC], F32, isOutput=True)
    logT = nc.declare_dram_parameter("logT", [VP2, TPC], BF16, isOutput=True)

    with tile.TileContext(nc) as tc, ExitStack() as ctx:
        p_h = ctx.enter_context(tc.tile_pool(name="p_h", bufs=1))
        p_hr = ctx.enter_context(tc.tile_pool(name="p_hr", bufs=1))
        p_act = ctx.enter_context(tc.tile_pool(name="p_act", bufs=2))
        p_tmp = ctx.enter_context(tc.tile_pool(name="p_tmp", bufs=1))
        p_g = ctx.enter_context(tc.tile_pool(name="p_g", bufs=1))
        p_w = ctx.enter_context(tc.tile_pool(name="p_w", bufs=10))
        p_wB = ctx.enter_context(tc.tile_pool(name="p_wB", bufs=6))
        p_lo = ctx.enter_context(tc.tile_pool(name="p_lo", bufs=3))
        p_st = ctx.enter_context(tc.tile_pool(name="p_st", bufs=2))
        p_c = ctx.enter_context(tc.tile_pool(name="p_c", bufs=1))
        p_mm = ctx.enter_context(tc.tile_pool(name="p_mm", bufs=5, space="PSUM"))
        p_bc = ctx.enter_context(tc.tile_pool(name="p_bc", bufs=2, space="PSUM"))
        p_s12 = ctx.enter_context(tc.tile_pool(name="p_s12", bufs=1, space="PSUM"))

        ones128f = p_c.tile([128, 1], F32, tag="ones128f")
        nc.gpsimd.memset(ones128f[:], 1.0)
        ones128b = p_c.tile([128, 1], BF16, tag="ones128b")
        nc.vector.tensor_copy(ones128b[:], ones128f[:])
        eps_t = p_c.tile([1, 1], F32, tag="eps")
        nc.gpsimd.memset(eps_t[:], LN_EPS)
        rowf = p_c.tile([1, 128], F32, tag="rowf")
        nc.gpsimd.memset(rowf[:], 1.0)
        onesrow = p_c.tile([1, 128], F32R, tag="onesrow")  # stationary for bcasts
        nc.vector.tensor_copy(onesrow[:], rowf[:])


        h_fm = p_h.tile([128, 8, TPC], F32, tag="h")
        hr = p_hr.tile([128, 8, TPC], BF16, tag="hr")
        hsq = p_hr.tile([128, 8, TPC], BF16, tag="hsq")
        # chunked load + immediate bf16 shadow prep (per k, per half);
        # issued before the bias loads so the first LN isn't queued behind them
        for k in range(8):
            for ci, cs in enumerate(HALVES):
                eng = nc.gpsimd if (k * 2 + ci) % 2 == 0 else nc.sync
                eng.dma_start(h_fm[:, k, cs], hT[k * 128 : (k + 1) * 128, cs])
                nc.vector.tensor_copy(hr[:, k, cs], h_fm[:, k, cs])
                nc.scalar.activation(hsq[:, k, cs], h_fm[:, k, cs], AF.Square)

        b1_sb = {}
        b2_sb = {}
        catt_sb = {}
        for li in range(2):
            t1 = p_c.tile([128, DFF // 128], F32, tag=f"b1_{li}")
            nc.sync.dma_start(t1[:], b1d[li])
            b1_sb[li] = t1
            t2 = p_c.tile([128, DIM // 128], F32, tag=f"b2_{li}")
            nc.sync.dma_start(t2[:], b2d[li])
            b2_sb[li] = t2
            t3 = p_c.tile([128, DIM // 128], F32, tag=f"catt_{li}")
            nc.sync.dma_start(t3[:], cattd[li])
            catt_sb[li] = t3

        def update_shadow(m, last=False):
            """After h_fm[:, m, :] residual update: refresh hr (+hsq/store).

            hr cast on vector right behind the residual stt; hsq on scalar."""
            nc.vector.tensor_copy(hr[:, m, :], h_fm[:, m, :])
            if last:
                for cs in HALVES:
                    nc.sync.dma_start(hbT[m * 128 : (m + 1) * 128, cs], h_fm[:, m, cs])
            else:
                nc.scalar.activation(hsq[:, m, :], h_fm[:, m, :], AF.Square)

        def emit_ln():
            """x_hat = (h - mu) * rstd -> returns bf16 act tile [128, 8, TPC]."""
            s12 = p_s12.tile([33, TPC], F32, tag="s12")
            for cs in HALVES:
                for k in range(8):
                    nc.tensor.matmul(
                        s12[0:1, cs], ones128b[:], hr[:, k, cs],
                        start=(k == 0), stop=(k == 7),
                    )
                for k in range(8):
                    nc.tensor.matmul(
                        s12[32:33, cs], ones128b[:], hsq[:, k, cs],
                        start=(k == 0), stop=(k == 7),
                    )
            mu_t = p_st.tile([1, TPC], F32, tag="mu")
            var_t = p_st.tile([1, TPC], F32, tag="var")
            rstd_t = p_st.tile([1, TPC], F32R, tag="rstd")
            bt_t = p_st.tile([1, TPC], F32R, tag="bt")
            abA = p_bc.tile([128, TPC], F32, tag="bc", name="abA")
            bbB = p_bc.tile([128, TPC], F32, tag="bc", name="bbB")
            # bf16 copies of the broadcast rows: the apply then runs all-16-bit
            # on the DVE (2x throughput) reading the bf16 shadow hr
            abA_b = p_st.tile([128, TPC], BF16, tag="abA_b")
            bbB_b = p_st.tile([128, TPC], BF16, tag="bbB_b")
            with nc.allow_low_precision(reason="LN rows feed f32r matmuls"):
                for cs in HALVES:
                    nc.vector.tensor_scalar_mul(mu_t[:, cs], s12[0:1, cs], 1.0 / DIM)
                    # E[x^2] on the scalar engine, in parallel with mu
                    nc.scalar.activation(
                        var_t[:, cs], s12[32:33, cs], AF.Copy, scale=1.0 / DIM
                    )
                    musq = p_st.tile([1, TPC], F32, tag="musq")
                    nc.vector.scalar_tensor_tensor(
                        musq[:, cs], mu_t[:, cs], -1.0, mu_t[:, cs], OP.mult, OP.mult
                    )
                    nc.vector.tensor_add(var_t[:, cs], var_t[:, cs], musq[:, cs])
                    # var+eps > 0, so 1/sqrt(|x|) == rsqrt
                    nc.scalar.activation(
                        rstd_t[:, cs], var_t[:, cs], AF.Abs_reciprocal_sqrt,
                        bias=eps_t[:], scale=1.0,
                    )
                    nc.vector.scalar_tensor_tensor(
                        bt_t[:, cs], mu_t[:, cs], -1.0, rstd_t[:, cs],
                        OP.mult, OP.mult,
                    )
                    nc.tensor.matmul(
                        abA[:, cs], onesrow[:], rstd_t[:, cs], start=True, stop=True
                    )
                    nc.tensor.matmul(
                        bbB[:, cs], onesrow[:], bt_t[:, cs], start=True, stop=True
                    )
                    # casts split across engines so they overlap
                    nc.scalar.activation(abA_b[:, cs], abA[:, cs], AF.Copy)
                    nc.vector.tensor_copy(bbB_b[:, cs], bbB[:, cs])
            dst = p_act.tile([128, 8, TPC], BF16, tag="act")
            with nc.allow_low_precision(reason="bf16 matmul inputs"):
                for k in range(8):
                    nc.vector.tensor_mul(dst[:, k, :], hr[:, k, :], abA_b[:])
                    nc.vector.tensor_add(dst[:, k, :], dst[:, k, :], bbB_b[:])
            return dst

        def matmul_stream(src_fm, wdram, kt, mt, epilogue,
                          groups=None, m_major_last=False, split_first_cols=False):
            """dst[m] = sum_k w[k,m].T @ src[k], feature-major, full 512 moving.

            src_fm: [128, kt, TPC] bf16; wdram: [kt*128, mt*128] bf16.
            epilogue(m, acc) consumes the accumulated PSUM tile.
            groups: m-tile group sizes (default fours). m_major_last runs the
            last group m-at-a-time so its epilogues stagger instead of all
            releasing at stream end (keeps the next LN's stats fed).
            """
            if groups is None:
                groups = [4] * (mt // 4) + ([mt % 4] if mt % 4 else [])
            m0 = 0
            for gi, gsz in enumerate(groups):
                w_ = gsz * 128
                m_major = m_major_last and gi == len(groups) - 1 and kt <= 8
                if gi == 0 and split_first_cols and kt <= 8:
                    # half-width column passes (h0 then h1) as SEQUENTIAL
                    # PSUM region groups in the same banks: the h0 pass only
                    # needs half the LN apply, hiding the h1 chain tail
                    wts = []
                    for k in range(kt):
                        wt = p_w.tile([128, 512], BF16, tag="wt")
                        nc.sync.dma_start(
                            wt[:, :w_],
                            wdram[k * 128 : (k + 1) * 128, m0 * 128 : m0 * 128 + w_],
                        )
                        wts.append(wt)
                    accs = {
                        ml: p_mm.tile([128, TPC], F32, tag="mm", name=f"acc{ml}")
                        for ml in range(gsz)
                    }
                    for cs in HALVES:
                        for k in range(kt):
                            for ml in range(gsz):
                                nc.tensor.matmul(
                                    accs[ml][:, cs],
                                    wts[k][:, ml * 128 : (ml + 1) * 128],
                                    src_fm[:, k, cs],
                                    start=(k == 0),
                                    stop=(k == kt - 1),
                                )
                    for ml in range(gsz):
                        epilogue(m0 + ml, accs[ml])
                    m0 += gsz
                    continue
                if m_major:
                    wts = []
                    for k in range(kt):
                        wt = p_w.tile([128, 512], BF16, tag="wt")
                        nc.sync.dma_start(
                            wt[:, :w_],
                            wdram[k * 128 : (k + 1) * 128, m0 * 128 : m0 * 128 + w_],
                        )
                        wts.append(wt)
                    for ml in range(gsz):
                        acc = p_mm.tile([128, TPC], F32, tag="mm", name=f"acc{ml}")
                        for k in range(kt):
                            nc.tensor.matmul(
                                acc[:],
                                wts[k][:, ml * 128 : (ml + 1) * 128],
                                src_fm[:, k, :],
                                start=(k == 0),
                                stop=(k == kt - 1),
                            )
                        epilogue(m0 + ml, acc)
                else:
                    accs = {}
                    for k in range(kt):
                        wt = p_w.tile([128, 512], BF16, tag="wt")
                        nc.sync.dma_start(
                            wt[:, :w_],
                            wdram[k * 128 : (k + 1) * 128, m0 * 128 : m0 * 128 + w_],
                        )
                        for ml in range(gsz):
                            if k == 0:
                                accs[ml] = p_mm.tile(
                                    [128, TPC], F32, tag="mm", name=f"acc{ml}"
                                )
                            nc.tensor.matmul(
                                accs[ml][:],
                                wt[:, ml * 128 : (ml + 1) * 128],
                                src_fm[:, k, :],
                                start=(k == 0),
                                stop=(k == kt - 1),
                            )
                    for ml in range(gsz):
                        epilogue(m0 + ml, accs[ml])
                m0 += gsz

        for li in range(2):
            # --- attention (seq len 1): h += LN1(h) @ wv' @ wo + c_att ---
            a_fm = emit_ln()
            tmp_fm = p_tmp.tile([128, 8, TPC], BF16, tag="tmp")

            def ep_tmp(m, acc):
                nc.vector.tensor_copy(tmp_fm[:, m, :], acc[:])

            matmul_stream(a_fm, wvf[li], 8, 8, ep_tmp)

            def ep_resid_att(m, acc, li=li):
                nc.vector.scalar_tensor_tensor(
                    h_fm[:, m, :], acc[:], catt_sb[li][:, m : m + 1], h_fm[:, m, :],
                    OP.add, OP.add,
                )
                update_shadow(m)

            matmul_stream(tmp_fm, wof[li], 8, 8, ep_resid_att)

            # --- mlp: h += gelu(LN2(h) @ w1' + b1') @ w2 + b2 ---
            m_fm = emit_ln()
            g_fm = p_g.tile([128, 32, TPC], BF16, tag="g")

            def ep_gelu(m, acc, li=li):
                nc.scalar.activation(
                    g_fm[:, m, :],
                    acc[:],
                    AF.Gelu_apprx_tanh,
                    bias=b1_sb[li][:, m : m + 1],
                    scale=1.0,
                )

            matmul_stream(m_fm, w1f[li], 8, 32, ep_gelu)

            last = li == 1

            def ep_resid_mlp(m, acc, li=li, last=last):
                nc.vector.scalar_tensor_tensor(
                    h_fm[:, m, :], acc[:], b2_sb[li][:, m : m + 1], h_fm[:, m, :],
                    OP.add, OP.add,
                )
                update_shadow(m, last=last)

            matmul_stream(g_fm, w2f[li], 32, 8, ep_resid_mlp)

        # --- head: logits[v, t] = head_w[v, :] @ hb[:, t], full vocab ---
        for mg in range((NVG + 3) // 4):
            mls = [ml for ml in range(4) if mg * 4 + ml < NVG]
            w_ = len(mls) * 128
            wtbs = []
            for kc in range(2):
                wtb = p_wB.tile([128, 4, 512], BF16, tag="wtb")
                nc.sync.dma_start(
                    wtb[:, :, :w_],
                    hwTd[
                        kc * 512 : (kc + 1) * 512, mg * 512 : mg * 512 + w_
                    ].rearrange("(k p) v -> p k v", p=128),
                )
                wtbs.append(wtb)
            accs = {}
            for k in range(8):
                wtb = wtbs[k // 4]
                for ml in mls:
                    if k == 0:
                        accs[ml] = p_mm.tile([128, TPC], F32, tag="mm", name=f"ha{ml}")
                    nc.tensor.matmul(
                        accs[ml][:],
                        wtb[:, k % 4, ml * 128 : (ml + 1) * 128],
                        hr[:, k, :],
                        start=(k == 0),
                        stop=(k == 7),
                    )
            lo = p_lo.tile([128, 4, TPC], BF16, tag="lo")
            with nc.allow_low_precision(reason="bf16 logits output"):
                for ml in mls:
                    nc.vector.tensor_copy(lo[:, ml, :], accs[ml][:])
            if mg >= (NVG + 3) // 4 - 3:
                # near the end: per-vocab-tile (and, for the last two groups,
                # per-token-half) stores spread the drain across queues so the
                # kernel doesn't tail-wait on one big DMA
                fine = mg >= (NVG + 3) // 4 - 2
                for ml in mls:
                    vg = mg * 4 + ml
                    if fine:
                        for cs in HALVES:
                            nc.sync.dma_start(
                                logT[vg * 128 : (vg + 1) * 128, cs], lo[:, ml, cs]
                            )
                    else:
                        nc.sync.dma_start(
                            logT[vg * 128 : (vg + 1) * 128, :], lo[:, ml, :]
                        )
            else:
                nc.sync.dma_start(
                    logT[mg * 512 : mg * 512 + w_, :].rearrange(
                        "(g p) t -> p g t", p=128
                    ),
                    lo[:, : len(mls), :],
                )

    nc.compile()
    return nc


def _get():
    if "F" not in _cache:
        _cache["F"] = _build_F()
    return _cache["F"]


# --------------------------------------------------------------------------
# Host side
# --------------------------------------------------------------------------

def _gelu_tanh(x):
    return 0.5 * x * (1.0 + np.tanh(0.7978845608028654 * (x + 0.044715 * x * x * x)))


def _host_block1(hb, inputs):
    """Block-1 layers (li=2,3) + head, fp32 numpy, for non-exiting tokens."""
    hb = hb.astype(np.float32)
    for li in (2, 3):
        mu = hb.mean(-1, keepdims=True, dtype=np.float32)
        var = hb.var(-1, keepdims=True, dtype=np.float32)
        a = (hb - mu) / np.sqrt(var + LN_EPS)
        a = a * inputs["ln1_s"][li] + inputs["ln1_b"][li]
        hb = hb + (a @ inputs["wv"][li]) @ inputs["wo"][li]
        mu = hb.mean(-1, keepdims=True, dtype=np.float32)
        var = hb.var(-1, keepdims=True, dtype=np.float32)
        m = (hb - mu) / np.sqrt(var + LN_EPS)
        m = m * inputs["ln2_s"][li] + inputs["ln2_b"][li]
        hb = hb + _gelu_tanh(m @ inputs["w1"][li] + inputs["b1"][li]) @ inputs["w2"][
            li
        ] + inputs["b2"][li]
    return hb @ np.asarray(inputs["head_w"], np.float32).T


def kernel(**inputs):
    x = np.asarray(inputs["x"]).reshape(-1).astype(np.int64)
    emb = np.asarray(inputs["emb"], dtype=np.float32)
    head_w = np.asarray(inputs["head_w"], dtype=np.float32)
    f32c = lambda k: np.ascontiguousarray(np.asarray(inputs[k], dtype=np.float32))

    h0 = emb[x]  # [T, DIM]

    wv = f32c("wv")[:2]
    wo = f32c("wo")[:2]
    w1 = f32c("w1")[:2]
    w2 = f32c("w2")[:2]
    ln1s, ln1b = f32c("ln1_s")[:2], f32c("ln1_b")[:2]
    ln2s, ln2b = f32c("ln2_s")[:2], f32c("ln2_b")[:2]
    b1, b2 = f32c("b1")[:2], f32c("b2")[:2]

    bf = lambda a: np.ascontiguousarray(a).astype(ml_dtypes.bfloat16)
    wvf = bf(ln1s[:, :, None] * wv)                       # fold ln1 scale
    wof = bf(wo)
    w1f = bf(ln2s[:, :, None] * w1)                       # fold ln2 scale
    w2f = bf(w2)
    catt = np.einsum("ld,ldm->lm", ln1b, wv, optimize=True)
    catt = np.einsum("ld,ldm->lm", catt, wo, optimize=True).astype(np.float32)
    b1f = (b1 + np.einsum("ld,ldm->lm", ln2b, w1, optimize=True)).astype(np.float32)
    # pre-transposed per-partition bias layouts: [L, 128, M]
    tp = lambda a, m: np.ascontiguousarray(
        a.reshape(2, m, 128).transpose(0, 2, 1).astype(np.float32)
    )
    hwT = np.zeros((DIM, VP2), ml_dtypes.bfloat16)
    hwT[:, :VOCAB] = head_w.T.astype(ml_dtypes.bfloat16)

    ncF = _get()
    wF = {
        "wvf": wvf, "wof": wof, "w1f": w1f, "w2f": w2f,
        "b1ft": tp(b1f, DFF // 128), "b2t": tp(b2, DIM // 128),
        "cattt": tp(catt, DIM // 128), "hwT": hwT,
    }
    in_maps = []
    for c in range(NCORES):
        m = dict(wF)
        m["hT"] = np.ascontiguousarray(h0[c * TPC : (c + 1) * TPC].T)
        in_maps.append(m)
    res = run_bass_kernel_spmd(
        ncF, in_maps, core_ids=list(range(NCORES)), trace=TRACE
    )
    if TRACE:
        LAST_EXEC_NS["F"] = res.exec_time_ns
        LAST_PROFILE["F"] = res

    out = np.empty((T, VOCAB), np.float32)
    for c in range(NCORES):
        L = res.results[c]["logT"]  # [VP2, TPC] bf16
        out[c * TPC : (c + 1) * TPC, :] = L[:VOCAB].T.astype(np.float32)
    hbT = np.concatenate(
        [res.results[c]["hbT"] for c in range(NCORES)], axis=1
    )  # [DIM, T]

    # host softmax stats (chunked): max_prob = exp(M - MHAT) / sum exp(l - MHAT)
    M = np.empty(T, np.float32)
    Z = np.empty(T, np.float32)
    for i in range(0, T, 256):
        chunk = out[i : i + 256]
        M[i : i + 256] = chunk.max(1)
        Z[i : i + 256] = np.exp(chunk - MHAT, dtype=np.float32).sum(
            1, dtype=np.float32
        )
    max_prob = np.exp(M - MHAT).astype(np.float32) / Z
    cont = ~(max_prob >= THRESH)
    if cont.any():
        idx = np.where(cont)[0]
        out[idx] = _host_block1(hbT.T[idx], inputs)

    return out.reshape(tuple(np.asarray(inputs["x"]).shape) + (VOCAB,))
